# revision 1
# baseline (speedup 1.0000x reference)
"""TRN2 Bass kernel for CrossOpLayerUTPM (upper-triangular pair products).

out[b, p=(i,j)] = x[b,i] * x[b,j] * (L[i,:]  .  L[j,:]),  P = N*(N-1)/2 pairs.

Strategy (8 NeuronCores, data parallel over batch):
  - diagonal decomposition: pair (i, i+d) for d = 1..255. Batch on the
    128 SBUF partitions, (d, i) flattened on the free axis.
  - per core: 4 batch tiles x 8 diagonal-chunks (32 diagonals each).
    Two fused bf16 tensor_tensor ops per (tile, chunk):
      t1[b,(d,i)]  = x[b,i] * x[b,i+d]      (stride-0 d-repeat x overlapping
                                             window access patterns)
      out[b,(d,i)] = t1 * S_bcast[(d,i)]    (S = L@L.T diagonals, host-prep)
  - bf16 output DMA (halves HBM write traffic); host reassembles the
    triangular pair order and casts to f32.
"""
import os
import numpy as np
import ml_dtypes
from contextlib import ExitStack

import jax
from jax.sharding import Mesh, PartitionSpec
from jax.experimental.shard_map import shard_map

import concourse.bass as bass
import concourse.bacc as bacc
import concourse.tile as tile
from concourse import mybir
from concourse.bass2jax import (
    _bass_exec_p,
    install_neuronx_cc_hook,
    partition_id_tensor,
)

F32 = mybir.dt.float32
BF16 = mybir.dt.bfloat16
BF16NP = ml_dtypes.bfloat16

B, NCOL = 4096, 256
NCORES = 8
BPC = B // NCORES            # 512 batch rows per core
NT = BPC // 128              # 4 partition tiles per core
XPC = 512                    # padded x columns (reads reach i+d <= 286)
NCH = 8                      # diagonal chunks
D0 = [1 + 32 * c for c in range(NCH)]
WC = [NCOL - d0 for d0 in D0]            # i-window width per chunk
OFF = np.concatenate([[0], np.cumsum([32 * w for w in WC])]).astype(np.int64)
TOTF = int(OFF[-1])                      # 36608 flattened (d,i) columns


def _build_nc(reps=1):
    nc = bacc.Bacc("TRN2", target_bir_lowering=False, debug=False)
    xp_in = nc.dram_tensor("xp", [BPC, XPC], BF16, kind="ExternalInput")
    s_in = nc.dram_tensor("sb", [128, TOTF], BF16, kind="ExternalInput")
    out_t = nc.dram_tensor("out", [BPC, TOTF], BF16, kind="ExternalOutput")

    with tile.TileContext(nc) as tc, ExitStack() as ctx:
        cpool = ctx.enter_context(tc.tile_pool(name="const", bufs=1))
        work = ctx.enter_context(tc.tile_pool(name="work", bufs=3))

        xt = []
        for t in range(NT):
            x_tile = cpool.tile([128, XPC], BF16, name=f"xt{t}")
            nc.sync.dma_start(out=x_tile[:, :], in_=xp_in[t * 128:(t + 1) * 128, :])
            xt.append(x_tile)
        s_all = cpool.tile([128, TOTF], BF16, name="s_all")
        nc.sync.dma_start(out=s_all[:, :], in_=s_in[:, :])

        for r in range(reps):
            for c in range(NCH):
                d0, w = D0[c], WC[c]
                fsz = 32 * w
                off = int(OFF[c])
                for t in range(NT):
                    x_tile = xt[t]
                    base = x_tile[:, :].offset
                    in0 = x_tile[:, 0:w].unsqueeze(1).broadcast_to([128, 32, w])
                    in1 = bass.AP(x_tile.tensor, base + d0,
                                  [[XPC, 128], [1, 32], [1, w]])
                    t1 = work.tile([128, fsz], BF16, tag="t1",
                                   name=f"t1_{r}_{c}_{t}", bufs=3)
                    nc.vector.tensor_tensor(
                        t1[:, :].rearrange("p (d w) -> p d w", d=32),
                        in0, in1, op=mybir.AluOpType.mult)
                    o_sb = work.tile([128, fsz], BF16, tag="osb",
                                     name=f"o_{r}_{c}_{t}", bufs=3)
                    nc.vector.tensor_mul(o_sb[:, :], t1[:, :],
                                         s_all[:, off:off + fsz])
                    nc.sync.dma_start(
                        out=out_t[t * 128:(t + 1) * 128, off:off + fsz],
                        in_=o_sb[:, :])

    nc.compile()
    return nc


class _Runner:
    def __init__(self, nc, n_cores=NCORES):
        install_neuronx_cc_hook()
        self.nc = nc
        self.n_cores = n_cores
        partition_name = (
            nc.partition_id_tensor.name if nc.partition_id_tensor else None
        )
        in_names, out_names, out_avals, zero_outs = [], [], [], []
        for alloc in nc.m.functions[0].allocations:
            if not isinstance(alloc, mybir.MemoryLocationSet):
                continue
            name = alloc.memorylocations[0].name
            if alloc.kind == "ExternalInput":
                if name != partition_name:
                    in_names.append(name)
            elif alloc.kind == "ExternalOutput":
                shape = tuple(alloc.tensor_shape)
                dtype = mybir.dt.np(alloc.dtype)
                out_avals.append(jax.core.ShapedArray(shape, dtype))
                zero_outs.append(np.zeros(shape, dtype))
                out_names.append(name)
        self.n_params = len(in_names)
        self.param_names = list(in_names)
        self.out_names = out_names
        self.out_avals = out_avals
        self.zero_outs = zero_outs
        all_in = in_names + out_names
        if partition_name is not None:
            all_in.append(partition_name)

        def _body(*args):
            operands = list(args)
            if partition_name is not None:
                operands.append(partition_id_tensor())
            return tuple(_bass_exec_p.bind(
                *operands,
                out_avals=tuple(out_avals),
                in_names=tuple(all_in),
                out_names=tuple(out_names),
                lowering_input_output_aliases=(),
                sim_require_finite=False,
                sim_require_nnan=False,
                nc=nc,
            ))

        devices = jax.devices()[:n_cores]
        mesh = Mesh(np.asarray(devices), ("core",))
        n_outs = len(out_names)
        in_specs = (PartitionSpec("core"),) * (self.n_params + n_outs)
        out_specs = (PartitionSpec("core"),) * n_outs
        self.fn = jax.jit(
            shard_map(_body, mesh=mesh, in_specs=in_specs,
                      out_specs=out_specs, check_rep=False),
            keep_unused=True,
        )

    def run_concat(self, concat_in):
        concat_zeros = [
            np.zeros((self.n_cores * z.shape[0], *z.shape[1:]), z.dtype)
            for z in self.zero_outs
        ]
        outs = self.fn(*concat_in, *concat_zeros)
        return [np.asarray(o) for o in outs]


_CACHE = {}


def _get_runner(reps=1):
    if reps not in _CACHE:
        _CACHE[reps] = _Runner(_build_nc(reps))
    return _CACHE[reps]


def _host_prep(x, latent_emb):
    """Per-core inputs: padded bf16 x shards + broadcast diagonal-S."""
    x = np.asarray(x, np.float32)
    L = np.asarray(latent_emb, np.float32)
    s = L @ L.T                                          # [256, 256]

    s_flat = np.zeros(TOTF, np.float32)
    for c in range(NCH):
        d0, w = D0[c], WC[c]
        dd, ii = np.meshgrid(np.arange(32), np.arange(w), indexing="ij")
        d = d0 + dd
        j = ii + d
        valid = j <= NCOL - 1
        block = np.zeros((32, w), np.float32)
        iv = ii[valid]
        jv = j[valid]
        block[valid] = s[iv, jv]
        s_flat[OFF[c]:OFF[c + 1]] = block.reshape(-1)
    s_bcast = np.broadcast_to(
        s_flat.astype(BF16NP), (128, TOTF)).copy()

    xp_cores = []
    for c in range(NCORES):
        xs = x[c * BPC:(c + 1) * BPC]
        xp = np.zeros((BPC, XPC), BF16NP)
        xp[:, :NCOL] = xs.astype(BF16NP)
        xp_cores.append(xp)
    return xp_cores, s_bcast


_IDX = None


def _pair_index():
    """Map triu pair p=(i,j) -> flattened device (d,i) column."""
    global _IDX
    if _IDX is None:
        iu, ju = np.triu_indices(NCOL, k=1)
        d = ju - iu
        c = (d - 1) // 32
        wc = np.array(WC)[c]
        dd = (d - 1) % 32
        _IDX = (OFF[c] + dd * wc + iu).astype(np.int64)
    return _IDX


def kernel(x, latent_emb):
    xp_cores, s_bcast = _host_prep(x, latent_emb)
    runner = _get_runner(1)

    concat_in = []
    for name in runner.param_names:
        if name == "xp":
            concat_in.append(np.concatenate(xp_cores, axis=0))
        elif name == "sb":
            concat_in.append(np.concatenate([s_bcast] * NCORES, axis=0))
        else:
            raise KeyError(name)
    outs = runner.run_concat(concat_in)
    dev = outs[runner.out_names.index("out")]            # [8*512, TOTF] bf16
    idx = _pair_index()
    return dev[:, idx].astype(np.float32)
